# revision 47
# baseline (speedup 1.0000x reference)
"""
AM-Softmax + intra-class loss kernel for Trainium2, 8 NeuronCores.

Strategy (class-sharded distributed softmax, fp8 matmuls, 3-engine exp):
  * Classes C=20000 sharded 2500/core (padded 2560 = 20 x 128 chunks).
    Matmuls run in fp8e4m3 with DoubleRow perf mode (contract 256 in one
    pass, 0.5 cyc/row): E.T and W arrive as host-cast fp8 (raw values; the
    normalisations are folded in on device, so casting is pure data
    movement).
  * Row/class norms: ||q8(e)||^2 and ||q8(w)||^2 are computed in the
    "flipped" layout - elementwise squares on DVE (2x mode) followed by
    PE ones-matmuls that contract the partition axis - then bounced
    DRAM->SBUF to land as per-partition scalars.  No ACT Square ops at
    all: ACT runs ONLY Exp (one LUT set).
  * Main loop per 128-row chunk: psum A [128,1536] (chunks 0-11) is
    exp'd by ACT (scale=1/||e||, bias=-30, fused row-sum accumulator);
    psum B [128,1024] (chunks 12-19) is drained by a DVE Schraudolph
    fast-exp (t = z*(rinv*c1) + c2 -> int16, bitcast bf16), then summed
    by a Pool bf16 pairwise add + DVE add + short reduce.  Pad classes
    get zero int16 codes (= +0.0 bf16) so they vanish from the sums.
    The last 2 chunks' B tiles are exp'd by ACT directly (its exp
    stream ends before the DVE/Pool pipeline would drain).
  * cos <= 1 so s*cos <= 30 is a fixed logsumexp offset (no max pass).
  * Label logits: host gathers W[labels] rows and ships er/wl
    TRANSPOSED; device computes ||er+wl||^2, ||er||^2, ||wl||^2 via ACT
    squares + ones-matmuls; host recovers the dot and cos (polarised).
  * Intra-class term: sum_{i<j}(1 - e_i.e_j) = 28 - (||sum_g e||^2-8)/2
    via one selection-matmul per core; host combines in float64.
"""

import numpy as np
import ml_dtypes

import concourse.bacc as bacc
import concourse.bass as bass
import concourse.tile as tile
from concourse import mybir
from concourse.bass_utils import run_bass_kernel_spmd
from concourse.masks import make_identity

B = 4096
D = 256
C = 20000
G = 512
NSAMP = 8
NCORES = 8
CREAL = C // NCORES          # 2500 real classes per core
CSH = 2560                   # padded classes per core (20 x 128)
WCH = CSH // 128             # 20 class chunks
RCH = B // 128               # 32 row chunks
RPC = B // NCORES            # 512 rows per core (label path)
GPC = G // NCORES            # 64 groups per core
CA = 1536                    # ACT share (chunks 0-11)
NB = 2                       # B psum tiles of 512 (chunks 12-19)

AM_MARGIN = 0.3
AM_SCALE = 30.0
INTRA_MARGIN = 0.5
LAMBDA_INTRA = 0.1
OFF = 30.0

# Schraudolph fast-exp constants (bf16 target: 8 exp bits, 7 mantissa bits)
C1 = 128.0 * 1.4426950408889634          # 184.66496...
BADJ = 7.25                              # tuned to zero the mean ratio error
C2 = 16256.0 - BADJ - OFF * C1           # additive code constant

F32 = mybir.dt.float32
F8 = mybir.dt.float8e4
BF16 = mybir.dt.bfloat16
I16 = mybir.dt.int16
I32 = mybir.dt.int32
AF = mybir.ActivationFunctionType
ALU = mybir.AluOpType
AXL = mybir.AxisListType
DR = mybir.MatmulPerfMode.DoubleRow

# DVE-share columns of the second B tile (rest goes to Pool)
DVE_B2 = 0   # DVE takes B1 fully; Pool takes B2's real 452 cols


def build_program():
    nc = bacc.Bacc("TRN2", target_bir_lowering=False)

    et8_d = nc.dram_tensor("et8", [128, 2, B], F8, kind="ExternalInput")
    w8_d = nc.dram_tensor("w8", [128, WCH, D], F8, kind="ExternalInput")
    wt8_d = nc.dram_tensor("wt8", [128, 2, CSH], F8, kind="ExternalInput")
    er_d = nc.dram_tensor("er", [128, 2, RPC], BF16, kind="ExternalInput")
    wl_d = nc.dram_tensor("wl", [128, 2, RPC], BF16, kind="ExternalInput")
    eg_d = nc.dram_tensor("eg", [128, 4, D], BF16, kind="ExternalInput")
    sel_d = nc.dram_tensor("sel", [128, GPC], BF16, kind="ExternalInput")

    esq_scr = nc.dram_tensor("esq_scr", [8, 512], F32, kind="Internal")
    wsq_scr = nc.dram_tensor("wsq_scr", [5, 512], F32, kind="Internal")

    out_s = nc.dram_tensor("out_s", [128, RCH], F32, kind="ExternalOutput")
    out_lc = nc.dram_tensor("out_lc", [3, RPC], F32, kind="ExternalOutput")
    out_iv = nc.dram_tensor("out_iv", [GPC, 1], F32, kind="ExternalOutput")

    from contextlib import ExitStack
    with tile.TileContext(nc) as tc, ExitStack() as ctx:
        big = ctx.enter_context(tc.tile_pool(name="big", bufs=1))
        scr = ctx.enter_context(tc.tile_pool(name="scr", bufs=2))

        NWT = 32

        def rsqrt_dve(dst, x, n, scale=1.0):
            """dst[:, :n] = scale/sqrt(x[:, :n]) DVE-only Newton rsqrt."""
            yi = scr.tile([128, NWT], I32, tag="nwty")
            nc.vector.tensor_scalar(out=yi[:, :n], in0=x.bitcast(I32),
                                    scalar1=1, scalar2=None,
                                    op0=ALU.arith_shift_right)
            nc.vector.tensor_scalar(out=yi[:, :n], in0=yi[:, :n],
                                    scalar1=-1, scalar2=None,
                                    op0=ALU.bitwise_xor)
            nc.vector.tensor_scalar(out=yi[:, :n], in0=yi[:, :n],
                                    scalar1=0x5f3759e0, scalar2=None,
                                    op0=ALU.add)
            y = yi.bitcast(F32)
            t = scr.tile([128, NWT], F32, tag="nwtt")
            for it in range(3):
                nc.vector.tensor_mul(t[:, :n], y[:, :n], y[:, :n])
                nc.vector.tensor_mul(t[:, :n], t[:, :n], x)
                last = it == 2
                nc.vector.tensor_scalar(
                    out=t[:, :n], in0=t[:, :n],
                    scalar1=(-0.5 * scale) if last else -0.5,
                    scalar2=(1.5 * scale) if last else 1.5,
                    op0=ALU.mult, op1=ALU.add)
                nc.vector.tensor_mul(dst if last else y[:, :n], y[:, :n],
                                     t[:, :n])

        # ---------------- input DMAs ----------------------------------------
        w8 = big.tile([128, WCH, D], F8)
        wt8 = big.tile([128, 2, CSH], F8)
        et8 = big.tile([128, 2, B], F8)
        ersb = big.tile([128, 2, RPC], BF16)
        wlsb = big.tile([128, 2, RPC], BF16)
        egsb = big.tile([128, 4, D], BF16)
        selsb = big.tile([128, GPC], BF16)

        for q4 in range(4):
            sli = slice(q4 * 640, (q4 + 1) * 640)
            nc.sync.dma_start(out=wt8[:, :, sli], in_=wt8_d[:, :, sli])
        nc.sync.dma_start(out=et8[:, :, 0:2048], in_=et8_d[:, :, 0:2048])
        nc.sync.dma_start(out=et8[:, :, 2048:4096], in_=et8_d[:, :, 2048:4096])
        nc.sync.dma_start(out=w8, in_=w8_d[:])

        ones_bf = big.tile([128, 1], BF16)
        nc.vector.memset(ones_bf, 1.0)
        identb = big.tile([128, 128], BF16)
        make_identity(nc, identb)
        identf = big.tile([128, 128], F32)
        make_identity(nc, identf)
        negoff = big.tile([128, 1], F32)
        nc.vector.memset(negoff, -OFF)
        # force the Exp LUT table load off the critical path
        warmup = big.tile([128, 1], BF16)
        nc.scalar.activation(out=warmup, in_=negoff, func=AF.Exp)

        esq = big.tile([128, RCH], F32)
        wsq = big.tile([128, WCH], F32)
        rinv = big.tile([128, RCH], F32)
        rinvc1 = big.tile([128, RCH], F32)
        winv = big.tile([128, WCH], F32)
        WT8 = big.tile([128, 2, CSH], F8)

        # ---------------- prep phase: norms via flipped ones-matmuls --------
        with tc.tile_pool(name="prep", bufs=1, space="PSUM") as pp, \
             tc.tile_pool(name="tp", bufs=2, space="PSUM") as tp:
            # wsq: wt8 o wt8 -> bf16, then ones-matmul per 512-block
            wsqsrc = big.tile([128, 2, CSH], BF16)
            for q4 in range(4):
                sli = slice(q4 * 640, (q4 + 1) * 640)
                nc.scalar.activation(out=wsqsrc[:, :, sli],
                                     in_=wt8[:, :, sli], func=AF.Square)
            pw = []
            for i in range(2):
                pwt = pp.tile([128, 512], F32, tag=f"pw{i}")
                pw.append(pwt)
            for j in range(5):
                dst = pw[j // 3][(j % 3) * 32:(j % 3) * 32 + 1, :]
                for kd in range(2):
                    nc.tensor.matmul(dst, lhsT=ones_bf,
                                     rhs=wsqsrc[:, kd, j * 512:(j + 1) * 512],
                                     start=(kd == 0), stop=(kd == 1))
            for i, nrow in ((0, 3), (1, 2)):
                stg = scr.tile([128, 512], F32, tag="stage")
                nc.scalar.copy(out=stg[0:97], in_=pw[i][0:97])
                nc.sync.dma_start(
                    out=wsq_scr[3 * i:3 * i + nrow],
                    in_=stg.rearrange("(a b) f -> a b f", b=32)[0:nrow, 0])
            nc.sync.dma_start(out=wsq, in_=wsq_scr[:].rearrange(
                "a (b p) -> p (a b)", p=128))

            nc.sync.dma_start(out=egsb, in_=eg_d[:])
            nc.sync.dma_start(out=ersb, in_=er_d[:])
            nc.sync.dma_start(out=wlsb, in_=wl_d[:])
            nc.sync.dma_start(out=selsb, in_=sel_d[:])

            # winv first: it gates the whole W prep pipeline
            rsqrt_dve(winv, wsq, WCH, scale=float(AM_SCALE))

            # W normalise (x30) into bf16, PE transpose, ACT copy casts to fp8
            w8n = big.tile([128, WCH, D], BF16)
            for c in range(WCH):
                nc.vector.tensor_scalar_mul(w8n[:, c], w8[:, c],
                                            winv[:, c:c + 1])
                pt = tp.tile([128, 2, 128], BF16, tag="tp")
                for kd in range(2):
                    nc.tensor.transpose(pt[:, kd],
                                        w8n[:, c, kd * 128:(kd + 1) * 128],
                                        identb)
                if c % 2 == 0:
                    nc.scalar.copy(out=WT8[:, :, c * 128:(c + 1) * 128],
                                   in_=pt)
                else:
                    nc.vector.tensor_copy(out=WT8[:, :, c * 128:(c + 1) * 128],
                                          in_=pt)

            # esq: et8 o et8 -> bf16, ones-matmul per 512-block
            esqsrc = big.tile([128, 2, B], BF16)
            for q in range(8):
                sli = slice(q * 512, (q + 1) * 512)
                if q < 2:
                    for kd in range(2):
                        nc.vector.tensor_mul(esqsrc[:, kd, sli],
                                             et8[:, kd, sli], et8[:, kd, sli])
                else:
                    nc.scalar.activation(out=esqsrc[:, :, sli],
                                         in_=et8[:, :, sli], func=AF.Square)
            pe = []
            for i in range(3):
                pet = pp.tile([128, 512], F32, tag=f"pe{i}")
                pe.append(pet)
            for j in range(8):
                dst = pe[j // 3][(j % 3) * 32:(j % 3) * 32 + 1, :]
                for kd in range(2):
                    nc.tensor.matmul(dst, lhsT=ones_bf,
                                     rhs=esqsrc[:, kd, j * 512:(j + 1) * 512],
                                     start=(kd == 0), stop=(kd == 1))
            for i, nrow in ((0, 3), (1, 3), (2, 2)):
                stg = scr.tile([128, 512], F32, tag="stage")
                nc.scalar.copy(out=stg[0:97], in_=pe[i][0:97])
                nc.sync.dma_start(
                    out=esq_scr[3 * i:3 * i + nrow],
                    in_=stg.rearrange("(a b) f -> a b f", b=32)[0:nrow, 0])
            nc.sync.dma_start(out=esq, in_=esq_scr[:].rearrange(
                "a (b p) -> p (a b)", p=128))

            rsqrt_dve(rinv, esq, RCH)
            nc.vector.tensor_scalar(out=rinvc1, in0=rinv, scalar1=float(C1),
                                    scalar2=None, op0=ALU.mult)

            # label path: tt/ersq/wlsq via flipped ones-matmuls
            #   ||er+wl||^2, ||er||^2, ||wl||^2 per row -> host gets tt
            ewsum = big.tile([128, 2, RPC], BF16)
            nc.vector.tensor_add(ewsum, ersb, wlsb)
            lcsq = big.tile([128, 3, 2, RPC], BF16)
            for li, srct in ((0, ewsum), (1, ersb), (2, wlsb)):
                nc.scalar.activation(out=lcsq[:, li], in_=srct,
                                     func=AF.Square)
            plc = pp.tile([128, RPC], F32, tag="plc")
            for li in range(3):
                dst = plc[li * 32:li * 32 + 1, :]
                for kd in range(2):
                    nc.tensor.matmul(dst, lhsT=ones_bf,
                                     rhs=lcsq[:, li, kd],
                                     start=(kd == 0), stop=(kd == 1))
            stlc = scr.tile([128, RPC], F32, tag="stage")
            nc.scalar.copy(out=stlc[0:65], in_=plc[0:65])
            nc.sync.dma_start(
                out=out_lc[:],
                in_=stlc.rearrange("(a b) f -> a b f", b=32)[0:3, 0])


        # ---------------- main loop ----------------------------------------
        asums = big.tile([128, RCH], F32)
        bsums = big.tile([128, RCH], F32)

        # two persistent code buffers; zero the 60 pad columns once
        code_tiles = []
        for ci in range(2):
            ct = big.tile([128, 1024], I16, tag=f"codes{ci}")
            nc.vector.memset(ct[:, 964:1024], 0)
            code_tiles.append(ct)

        DVE_COLS = 512   # DVE drains B1 fully; Pool drains B2

        with tc.tile_pool(name="pA", bufs=2, space="PSUM") as pA, \
             tc.tile_pool(name="pB", bufs=1, space="PSUM") as pB:
            def emit_pa(r):
                lhs = et8[:, :, r * 128:(r + 1) * 128]
                pa = pA.tile([128, CA], F32, tag="mma")
                for tb in range(3):
                    nc.tensor.matmul(pa[:, tb * 512:(tb + 1) * 512],
                                     lhsT=lhs,
                                     rhs=WT8[:, :, tb * 512:(tb + 1) * 512],
                                     start=True, stop=True, perf_mode=DR)
                return pa

        # pa_{r+1} is emitted before pb_r so PE never stalls the ACT stream
            pa_tiles = {0: emit_pa(0)}
            for r in range(RCH):
                if r + 1 < RCH:
                    pa_tiles[r + 1] = emit_pa(r + 1)
                pa = pa_tiles.pop(r)
                lhs = et8[:, :, r * 128:(r + 1) * 128]
                ct = code_tiles[r % 2]
                pb = pB.tile([128, 1024], F32, tag="mmb")
                pb_last = pb
                for h in range(NB):
                    nc.tensor.matmul(
                        pb[:, h * 512:(h + 1) * 512], lhsT=lhs,
                        rhs=WT8[:, :, CA + h * 512:CA + (h + 1) * 512],
                        start=True, stop=True, perf_mode=DR)
                if r < RCH - 2:
                    nc.vector.tensor_scalar(
                        out=ct[:, 0:964], in0=pb[:, 0:964],
                        scalar1=rinvc1[:, r:r + 1], scalar2=float(C2),
                        op0=ALU.mult, op1=ALU.add)
                # ACT: exp + fused row-sum over the A share
                s1 = scr.tile([128, CA], BF16, tag="expscr")
                nc.scalar.activation(out=s1, in_=pa, func=AF.Exp,
                                     scale=rinv[:, r:r + 1],
                                     bias=negoff[:, 0:1],
                                     accum_out=asums[:, r:r + 1])
                if r >= RCH - 2:
                    # tail chunks: ACT exps the B tile directly (pipeline
                    # drain is shorter than the DVE/Pool handoff chain)
                    s1b = scr.tile([128, 964], BF16, tag="expscrb")
                    nc.scalar.activation(out=s1b, in_=pb[:, 0:964],
                                         func=AF.Exp,
                                         scale=rinv[:, r:r + 1],
                                         bias=negoff[:, 0:1],
                                         accum_out=bsums[:, r:r + 1])
                    continue
                # Pool does tree level 1, DVE level 2 + final reduce
                cb = ct.bitcast(BF16)
                t1 = scr.tile([128, 512], BF16, tag="tree1")
                nc.gpsimd.tensor_tensor(out=t1, in0=cb[:, 0:512],
                                        in1=cb[:, 512:1024], op=ALU.add)
                t2 = scr.tile([128, 256], BF16, tag="tree2")
                nc.vector.tensor_tensor(out=t2, in0=t1[:, 0:256],
                                        in1=t1[:, 256:512], op=ALU.add)
                nc.vector.tensor_reduce(out=bsums[:, r:r + 1], in_=t2,
                                        axis=AXL.X, op=ALU.add)

        sums = big.tile([128, RCH], F32)
        nc.vector.tensor_add(sums, asums, bsums)
        nc.sync.dma_start(out=out_s[:], in_=sums)

        # ---------------- tail: intra + label pieces ------------------------
        with tc.tile_pool(name="tail", bufs=1, space="PSUM") as tpp:
            # intra: normalise eg rows, selection-matmul, ||sum_g||^2
            egsq = big.tile([128, 4], F32)
            egs = scr.tile([128, 4, D], BF16, tag="egs")
            nc.gpsimd.tensor_mul(egs, egsb, egsb)
            nc.vector.tensor_reduce(out=egsq, in_=egs, axis=AXL.X, op=ALU.add)
            eginv = big.tile([128, 4], F32)
            rsqrt_dve(eginv, egsq, 4)
            for j in range(4):
                nc.vector.tensor_scalar_mul(egsb[:, j], egsb[:, j],
                                            eginv[:, j:j + 1])
            sg = tpp.tile([GPC, D], F32, tag="sg")
            for j in range(4):
                nc.tensor.matmul(sg, lhsT=selsb, rhs=egsb[:, j],
                                 start=(j == 0), stop=(j == 3))
            sgsb = scr.tile([GPC, D], BF16, tag="sgsb")
            nc.vector.tensor_copy(sgsb, sg)
            sgsq = scr.tile([GPC, D], BF16, tag="sgsq")
            nc.vector.tensor_mul(sgsq, sgsb, sgsb)
            ssq = big.tile([GPC, 1], F32)
            nc.vector.tensor_reduce(out=ssq, in_=sgsq, axis=AXL.X, op=ALU.add)
            npairs = NSAMP * (NSAMP - 1) / 2.0
            iv = big.tile([GPC, 1], F32)
            nc.vector.tensor_scalar(
                out=iv, in0=ssq,
                scalar1=-1.0 / (2.0 * npairs),
                scalar2=(1.0 - INTRA_MARGIN) + NSAMP / (2.0 * npairs),
                op0=ALU.mult, op1=ALU.add)
            nc.vector.tensor_scalar_max(iv, iv, 0.0)
            nc.sync.dma_start(out=out_iv[:], in_=iv)


    nc.finalize()
    return nc


def kernel(embeddings, labels, weight):
    e = np.ascontiguousarray(embeddings, dtype=np.float32)
    lab = np.asarray(labels).astype(np.int64)
    w = np.ascontiguousarray(weight, dtype=np.float32)
    assert e.shape == (B, D) and w.shape == (C, D) and lab.shape == (B,)

    members = np.argsort(lab, kind="stable").reshape(G, NSAMP)
    sel = np.tile(np.eye(GPC, dtype=np.float32), (2, 1)).astype(
        ml_dtypes.bfloat16)

    # host-side casts / layout moves (no arithmetic)
    et8_full = np.ascontiguousarray(e.T).astype(ml_dtypes.float8_e4m3)
    et8 = np.ascontiguousarray(et8_full.reshape(2, 128, B).transpose(1, 0, 2))

    in_maps = []
    for k in range(NCORES):
        wsh = np.zeros((CSH, D), np.float32)
        wsh[:CREAL] = w[k * CREAL:(k + 1) * CREAL]
        w8f = wsh.astype(ml_dtypes.float8_e4m3)
        w8 = np.ascontiguousarray(
            w8f.reshape(WCH, 128, D).transpose(1, 0, 2))
        wt8f = np.ascontiguousarray(w8f.T)              # [D, CSH]
        wt8 = np.ascontiguousarray(
            wt8f.reshape(2, 128, CSH).transpose(1, 0, 2))
        rows = slice(k * RPC, (k + 1) * RPC)
        erT = np.ascontiguousarray(e[rows].T).astype(ml_dtypes.bfloat16)
        wlT = np.ascontiguousarray(w[lab[rows]].T).astype(ml_dtypes.bfloat16)
        gm = members[k * GPC:(k + 1) * GPC]
        eg_idx = gm.T.reshape(-1)
        eg = np.ascontiguousarray(e[eg_idx]).astype(ml_dtypes.bfloat16)
        in_maps.append({
            "et8": et8, "w8": w8, "wt8": wt8,
            "er": np.ascontiguousarray(erT.reshape(2, 128, RPC).transpose(1, 0, 2)),
            "wl": np.ascontiguousarray(wlT.reshape(2, 128, RPC).transpose(1, 0, 2)),
            "eg": np.ascontiguousarray(eg.reshape(4, 128, D).transpose(1, 0, 2)),
            "sel": sel,
        })

    nc = build_program()
    res = run_bass_kernel_spmd(nc, in_maps, core_ids=list(range(NCORES)))
    global _last_results
    _last_results = res

    # ---------------- host combine (O(B), float64) -----------------------
    S = np.zeros(B, np.float64)
    for k in range(NCORES):
        S += res.results[k]["out_s"].T.reshape(B).astype(np.float64)
    cls = []
    for k in range(NCORES):
        pk = res.results[k]["out_lc"].astype(np.float64)
        ssq, ersq, wlsq = pk[0], pk[1], pk[2]
        tt = (ssq - ersq - wlsq) / 2.0
        cls.append(tt / np.sqrt(ersq * wlsq))
    cl = np.concatenate(cls)

    s, m = float(AM_SCALE), float(AM_MARGIN)
    S_adj = S - np.exp(s * cl - OFF) + np.exp(s * (cl - m) - OFF)
    am_i = (np.log(S_adj) + OFF) - s * (cl - m)
    am = am_i.mean()

    ivals = np.concatenate(
        [res.results[k]["out_iv"][:, 0] for k in range(NCORES)]
    ).astype(np.float64)
    intra = ivals.sum() / G
    total = am + LAMBDA_INTRA * intra
    return (np.float32(total), np.float32(am), np.float32(intra))
